# revision 1
# baseline (speedup 1.0000x reference)
"""Distributed Trainium2 (Bass/Tile) kernel for a causal self-attention block.

Reference computation (per batch b):
    qk = x_eps @ W_eps_attn ; q,k = split(qk) ; vp = v @ W_attn
    q,k = rope(q), rope(k)   (llama-style, 16 heads x 128 dims)
    y   = causal_softmax(q k^T / sqrt(128)) @ vp   (per head)
    v_out     = y @ W_proj
    x_eps_out = x_eps @ W_eps_proj

Sharding over 8 NeuronCores: core = (b, g) = 4-way batch x 2-way head-group
(8 heads per core).  W_eps_attn/W_attn are column-sharded by head; y is
exchanged pair-wise in two chunks (AllGather per 512-token half, pipelined
behind the second attention half and the independent x_eps_out projection)
and W_proj/W_eps_proj are used column-sharded so each core produces a
disjoint half of both outputs (no reduce needed).

All matmuls run in bf16 with fp32 PSUM accumulation.  Activations are
uploaded contraction-major (pre-transposed host-side, numerically identical
to a device-side transpose of the same bf16 data); the rotate-half RoPE
layout, the 1/sqrt(128) q-scale and all weight tiling/permutation are pure
host-side weight re-layout.
"""

import sys

sys.path.insert(0, "/opt/trn_rl_repo")

import numpy as np
import ml_dtypes

import concourse.bass as bass
import concourse.mybir as mybir
import concourse.tile as tile
from concourse import bacc
from concourse.bass_utils import run_bass_kernel_spmd

F32 = mybir.dt.float32
BF16 = mybir.dt.bfloat16
BF16_NP = ml_dtypes.bfloat16

B, T, DIM, H, HD = 4, 1024, 2048, 16, 128
NCORES = 8
HL = H // 2          # heads per core (8)
TT = T // 128        # t-tiles (8)
CC = DIM // 128      # contraction chunks (16)
QC = T // 512        # 512-wide q chunks (2)
COLS = DIM // 2      # local column count of each output half (1024)

_COMPILED = None
STAGE_MARKS = []


def _mark(nc, name):
    STAGE_MARKS.append((name, len(nc.inst_map)))


def _build():
    nc = bacc.Bacc(trn_type="TRN2", target_bir_lowering=False, debug=False,
                   num_devices=NCORES)

    # ---- per-core I/O (activations contraction-major, weights pre-packed) ----
    x_in = nc.dram_tensor("xT_bf", [CC, 128, T], BF16, kind="ExternalInput").ap()
    v_in = nc.dram_tensor("vT_bf", [CC, 128, T], BF16, kind="ExternalInput").ap()
    cos_in = nc.dram_tensor("cos_t", [64, T], BF16, kind="ExternalInput").ap()
    sin_in = nc.dram_tensor("sin_t", [64, T], BF16, kind="ExternalInput").ap()
    wqk_in = nc.dram_tensor("w_qk", [8, CC, 128, 256], BF16, kind="ExternalInput").ap()
    wat_in = nc.dram_tensor("w_attn", [CC, 128, COLS], BF16, kind="ExternalInput").ap()
    wpr_in = nc.dram_tensor("w_proj", [CC, 128, COLS], BF16, kind="ExternalInput").ap()
    wep_in = nc.dram_tensor("w_eps_proj", [CC, 128, COLS], BF16,
                            kind="ExternalInput").ap()
    v_out = nc.dram_tensor("v_out", [TT, 128, COLS], F32, kind="ExternalOutput").ap()
    x_out = nc.dram_tensor("x_out", [TT, 128, COLS], F32, kind="ExternalOutput").ap()

    # internal DRAM for the chunked pair-wise y exchange
    y_bounce = [nc.dram_tensor(f"y_bounce{qc}", [HL, 128, 512], BF16)
                for qc in range(QC)]
    y_gather = [nc.dram_tensor(f"y_gather{qc}", [2 * HL, 128, 512], BF16)
                for qc in range(QC)]

    with tile.TileContext(nc) as tc:
        with tc.tile_pool(name="persist", bufs=1) as pp, \
             tc.tile_pool(name="wstream", bufs=4) as wsp, \
             tc.tile_pool(name="ptile", bufs=4) as ptp, \
             tc.tile_pool(name="ropetmp", bufs=2) as rtp, \
             tc.tile_pool(name="outcp", bufs=3) as ocp, \
             tc.tile_pool(name="ps", bufs=8, space="PSUM") as psp:

            # all-ones stationary operand: the denominator matmul then lands
            # Z replicated on every PSUM partition (free row-broadcast)
            ones_mat = pp.tile([128, 128], BF16, tag="ones_mat")
            nc.vector.memset(ones_mat[:], 1.0)
            cosT = pp.tile([64, T], BF16, tag="cosT")
            nc.sync.dma_start(cosT[:], cos_in)
            sinT = pp.tile([64, T], BF16, tag="sinT")
            nc.sync.dma_start(sinT[:], sin_in)
            # causal 0/1 masks, variant m: keep (1) iff q_rel - k_rel - 128*m >= 0
            masks = []
            for m in range(4):
                mk = pp.tile([128, 512], BF16, tag=f"mask{m}")
                nc.gpsimd.memset(mk[:], 1.0)
                nc.gpsimd.affine_select(
                    out=mk[:], in_=mk[:], compare_op=mybir.AluOpType.is_ge,
                    fill=0.0, base=-128 * m, pattern=[[1, 512]],
                    channel_multiplier=-1)
                masks.append(mk)

            xT = [pp.tile([128, T], BF16, tag=f"xT{c}", name=f"xT{c}")
                  for c in range(CC)]
            # v_out projection weights are made resident early so the final
            # (collective-dependent) stage never waits on a weight stream
            wprT = [pp.tile([128, COLS], BF16, tag=f"wprT{c}", name=f"wprT{c}")
                    for c in range(CC)]

            with tc.tile_pool(name="vtpool", bufs=1) as vtp:
                vT = [vtp.tile([128, T], BF16, tag=f"vT{c}", name=f"vT{c}")
                      for c in range(CC)]

                with tc.tile_pool(name="qkpool", bufs=1) as qkp:
                    qT = [qkp.tile([128, T], BF16, tag=f"qT{j}", name=f"qT{j}")
                          for j in range(HL)]
                    kT = [qkp.tile([128, T], BF16, tag=f"kT{j}", name=f"kT{j}")
                          for j in range(HL)]
                    vp = [qkp.tile([128, COLS], BF16, tag=f"vp{t}", name=f"vp{t}")
                          for t in range(TT)]

                    # ---- stage B: q/k projection (transposed out) + RoPE ----
                    # 8 groups of 2 d-tiles -> 4 live PSUM accumulators/group.
                    # xT loads are interleaved with group 0's weight stream so
                    # TensorE starts ~immediately.
                    for G in range(8):
                        _mark(nc, f'B{G}')
                        psums = [[psp.tile([128, 512], F32, tag="ps",
                                           name=f"psB{G}_{i}_{q}")
                                  for q in range(QC)] for i in range(2)]
                        for c in range(CC):
                            if G == 0:
                                nc.sync.dma_start(xT[c][:], x_in[c])
                            wt = wsp.tile([128, 256], BF16, tag="wqk_s")
                            nc.sync.dma_start(wt[:], wqk_in[G, c])
                            for dl in range(2):
                                for qc in range(QC):
                                    nc.tensor.matmul(
                                        psums[dl][qc][:],
                                        wt[:, dl * 128:(dl + 1) * 128],
                                        xT[c][:, qc * 512:(qc + 1) * 512],
                                        start=(c == 0), stop=(c == CC - 1))
                        for dl in range(2):
                            dt = 2 * G + dl
                            dst = qT[dt] if dt < HL else kT[dt - HL]
                            for qc in range(QC):
                                ps = psums[dl][qc]
                                cs = slice(qc * 512, (qc + 1) * 512)
                                # split re/im into base-partition-0 tiles:
                                # walrus requires equal base partitions when
                                # both TensorTensor inputs live in SBUF
                                pre_re = rtp.tile([64, 512], BF16, tag="pre_re")
                                nc.vector.tensor_copy(pre_re[:], ps[0:64, :])
                                pre_im = rtp.tile([64, 512], BF16, tag="pre_im")
                                nc.vector.tensor_copy(pre_im[:], ps[64:128, :])
                                t1 = rtp.tile([64, 512], BF16, tag="rt1")
                                t2 = rtp.tile([64, 512], BF16, tag="rt2")
                                nc.vector.tensor_tensor(
                                    t1[:], pre_re[:], cosT[:, cs],
                                    mybir.AluOpType.mult)
                                nc.vector.tensor_tensor(
                                    t2[:], pre_im[:], sinT[:, cs],
                                    mybir.AluOpType.mult)
                                nc.vector.tensor_tensor(
                                    dst[0:64, cs], t1[:], t2[:],
                                    mybir.AluOpType.subtract)
                                t3 = rtp.tile([64, 512], BF16, tag="rt1")
                                t4 = rtp.tile([64, 512], BF16, tag="rt2")
                                nc.vector.tensor_tensor(
                                    t3[:], pre_re[:], sinT[:, cs],
                                    mybir.AluOpType.mult)
                                nc.vector.tensor_tensor(
                                    t4[:], pre_im[:], cosT[:, cs],
                                    mybir.AluOpType.mult)
                                nc.vector.tensor_tensor(
                                    dst[64:128, cs], t3[:], t4[:],
                                    mybir.AluOpType.add)

                    # vT arrives while stage B computes
                    for c in range(CC):
                        nc.sync.dma_start(vT[c][:], v_in[c])
                    for c in range(CC):
                        nc.sync.dma_start(wprT[c][:], wpr_in[c])

                    # ---- stages C+D interleaved per 512-token half: vp
                    # group, that half's attention, then its AllGather --
                    # the exchange for half 0 overlaps half 1's compute ----
                    for qc in range(QC):
                        _mark(nc, f'C{qc}')
                        tg = qc
                        psums = [[psp.tile([128, 512], F32, tag="ps",
                                           name=f"psC{tg}_{i}_{q}")
                                  for q in range(2)] for i in range(4)]
                        for c in range(CC):
                            wt = wsp.tile([128, COLS], BF16, tag="wat_s")
                            nc.sync.dma_start(wt[:], wat_in[c])
                            for tl in range(4):
                                t = tg * 4 + tl
                                for dh in range(2):
                                    nc.tensor.matmul(
                                        psums[tl][dh][:],
                                        vT[c][:, t * 128:(t + 1) * 128],
                                        wt[:, dh * 512:(dh + 1) * 512],
                                        start=(c == 0), stop=(c == CC - 1))
                        for tl in range(4):
                            t = tg * 4 + tl
                            for dh in range(2):
                                nc.vector.tensor_copy(
                                    vp[t][:, dh * 512:(dh + 1) * 512],
                                    psums[tl][dh][:])

                        _mark(nc, f'D{qc}')
                        for j in range(HL):
                            ktiles = 4 * qc + 4
                            py = psp.tile([128, 512], F32, tag="ps",
                                          name=f"py{j}_{qc}")
                            pz = psp.tile([128, 512], F32, tag="ps",
                                          name=f"pz{j}_{qc}")
                            for ki in range(ktiles):
                                pscr = psp.tile([128, 512], F32, tag="ps",
                                                name=f"pscr{j}_{qc}_{ki}")
                                nc.tensor.matmul(
                                    pscr[:],
                                    kT[j][:, ki * 128:(ki + 1) * 128],
                                    qT[j][:, qc * 512:(qc + 1) * 512],
                                    start=True, stop=True)
                                pt = ptp.tile([128, 512], BF16, tag="p_t")
                                nc.scalar.activation(
                                    pt[:], pscr[:],
                                    mybir.ActivationFunctionType.Exp)
                                if ki >= 4 * qc:
                                    nc.vector.tensor_tensor(
                                        pt[:], pt[:], masks[ki - 4 * qc][:],
                                        mybir.AluOpType.mult)
                                nc.tensor.matmul(
                                    py[:], vp[ki][:, j * 128:(j + 1) * 128], pt[:],
                                    start=(ki == 0), stop=(ki == ktiles - 1))
                                nc.tensor.matmul(
                                    pz[:], ones_mat[:], pt[:],
                                    start=(ki == 0), stop=(ki == ktiles - 1))
                            zr = rtp.tile([128, 512], F32, tag="zrec")
                            nc.vector.reciprocal(zr[:], pz[:])
                            yt = ptp.tile([128, 512], BF16, tag="y_t")
                            nc.vector.tensor_tensor(
                                yt[:], py[:], zr[:], mybir.AluOpType.mult)
                            nc.sync.dma_start(y_bounce[qc].ap()[j], yt[:])
                        # dispatch this half's exchange immediately
                        nc.gpsimd.collective_compute(
                            "AllGather", mybir.AluOpType.bypass,
                            replica_groups=[[0, 1], [2, 3], [4, 5], [6, 7]],
                            ins=[y_bounce[qc].ap()], outs=[y_gather[qc].ap()])

            def proj(w_of, lhs_fn, dst, tgs, psname):
                # out[t, :] += lhs[c]^T @ w[c] over 16 chunks; groups of 4 t-tiles
                for tg in tgs:
                    _mark(nc, f'P{psname}{tg}')
                    psums = [[psp.tile([128, 512], F32, tag="ps",
                                       name=f"ps{psname}{tg}_{i}_{q}")
                              for q in range(2)] for i in range(4)]
                    for c in range(CC):
                        wt = w_of(c)
                        for tl in range(4):
                            t = tg * 4 + tl
                            for dh in range(2):
                                nc.tensor.matmul(
                                    psums[tl][dh][:],
                                    lhs_fn(c, t),
                                    wt[:, dh * 512:(dh + 1) * 512],
                                    start=(c == 0), stop=(c == CC - 1))
                    for tl in range(4):
                        t = tg * 4 + tl
                        ot = ocp.tile([128, COLS], F32, tag="out_cp")
                        for dh in range(2):
                            nc.vector.tensor_copy(
                                ot[:, dh * 512:(dh + 1) * 512],
                                psums[tl][dh][:])
                        nc.sync.dma_start(dst[t], ot[:])

            def wep_stream(c):
                wt = wsp.tile([128, COLS], BF16, tag="wo_s")
                nc.sync.dma_start(wt[:], wep_in[c])
                return wt

            # tail interleave: the collective-independent x_eps_out groups are
            # spaced between the v_out groups so TensorE never idles (and never
            # cools to the 1.2 GHz HAM state) while an AllGather chunk is in
            # flight.  v_out rows [qc*512, ..+512) consume y columns of the
            # same token range, so group qc only needs exchange chunk qc.
            with tc.tile_pool(name="ygpool", bufs=1) as ygp:
                yG = [[ygp.tile([128, 512], BF16, tag=f"yG{qc}_{c}",
                                name=f"yG{qc}_{c}") for c in range(CC)]
                      for qc in range(QC)]
                for qc in range(QC):
                    _mark(nc, f"Xproj{qc}")
                    proj(wep_stream,
                         lambda c, t: xT[c][:, t * 128:(t + 1) * 128],
                         x_out, (qc,), "X")
                    for c in range(CC):
                        nc.sync.dma_start(yG[qc][c][:], y_gather[qc].ap()[c])
                    _mark(nc, f"Vproj{qc}")
                    proj(lambda c: wprT[c],
                         lambda c, t: yG[t // 4][c][:, (t % 4) * 128:
                                                    (t % 4) * 128 + 128],
                         v_out, (qc,), "V")

    nc.compile()
    return nc


def _prep_core_inputs(inputs):
    """Host-side shard prep: slicing, bf16 cast, weight permutation/packing."""
    x_eps = np.asarray(inputs["x_eps"], np.float32)
    v = np.asarray(inputs["v"], np.float32)
    cos = np.asarray(inputs["freqs_cos"], np.float32)
    sin = np.asarray(inputs["freqs_sin"], np.float32)
    Wqk = np.asarray(inputs["W_eps_attn"], np.float32)
    Wat = np.asarray(inputs["W_attn"], np.float32)
    Wpr = np.asarray(inputs["W_proj"], np.float32)
    Wep = np.asarray(inputs["W_eps_proj"], np.float32)

    cosT = np.ascontiguousarray(cos.T).astype(BF16_NP)     # [64, T]
    sinT = np.ascontiguousarray(sin.T).astype(BF16_NP)
    perm = np.concatenate([np.arange(0, HD, 2), np.arange(1, HD, 2)])  # rotate-half
    scale = np.float32(1.0 / np.sqrt(HD))

    # contraction-major (transposed) bf16 activations, tiled [CC, 128, T]
    xT_bf = [np.ascontiguousarray(x_eps[b].astype(BF16_NP).T)
             .reshape(CC, 128, T) for b in range(B)]
    vT_bf = [np.ascontiguousarray(v[b].astype(BF16_NP).T)
             .reshape(CC, 128, T) for b in range(B)]

    per_g = []
    for g in range(2):
        heads = range(g * HL, (g + 1) * HL)
        wq = np.concatenate(
            [Wqk[:, h * HD:(h + 1) * HD][:, perm] * scale for h in heads], axis=1)
        wk = np.concatenate(
            [Wqk[:, DIM + h * HD:DIM + (h + 1) * HD][:, perm] for h in heads],
            axis=1)
        wqk_cols = np.concatenate([wq, wk], axis=1)          # [2048, 2048]
        wqk_packed = np.empty((8, CC, 128, 256), np.float32)
        for G in range(8):
            for dl in range(2):
                dt = 2 * G + dl
                blk = wqk_cols[:, dt * 128:(dt + 1) * 128]    # [2048, 128]
                wqk_packed[G, :, :, dl * 128:(dl + 1) * 128] = \
                    blk.reshape(CC, 128, 128)
        cols = slice(g * COLS, (g + 1) * COLS)
        per_g.append({
            "w_qk": wqk_packed.astype(BF16_NP),
            "w_attn": Wat[:, cols].reshape(CC, 128, COLS).astype(BF16_NP),
            "w_proj": Wpr[:, cols].reshape(CC, 128, COLS).astype(BF16_NP),
            "w_eps_proj": Wep[:, cols].reshape(CC, 128, COLS).astype(BF16_NP),
        })

    in_maps = []
    for core in range(NCORES):
        b, g = divmod(core, 2)
        in_maps.append({
            "xT_bf": xT_bf[b],
            "vT_bf": vT_bf[b],
            "cos_t": cosT,
            "sin_t": sinT,
            **per_g[g],
        })
    return in_maps


def _get_compiled():
    global _COMPILED
    if _COMPILED is None:
        _COMPILED = _build()
    return _COMPILED


def kernel(**inputs):
    nc = _get_compiled()
    in_maps = _prep_core_inputs(inputs)
    res = run_bass_kernel_spmd(nc, in_maps, list(range(NCORES)))
    v_out = np.empty((B, T, DIM), np.float32)
    x_out = np.empty((B, T, DIM), np.float32)
    for core in range(NCORES):
        b, g = divmod(core, 2)
        r = res.results[core]
        cols = slice(g * COLS, (g + 1) * COLS)
        v_out[b][:, cols] = r["v_out"].reshape(T, COLS)
        x_out[b][:, cols] = r["x_out"].reshape(T, COLS)
    return (v_out, x_out)



# revision 2
# speedup vs baseline: 1.1743x; 1.1743x over previous
"""Distributed Trainium2 (Bass/Tile) kernel for a causal self-attention block.

Reference computation (per batch b):
    qk = x_eps @ W_eps_attn ; q,k = split(qk) ; vp = v @ W_attn
    q,k = rope(q), rope(k)   (llama-style, 16 heads x 128 dims)
    y   = causal_softmax(q k^T / sqrt(128)) @ vp   (per head)
    v_out     = y @ W_proj
    x_eps_out = x_eps @ W_eps_proj

Sharding over 8 NeuronCores: core = (b, g) = 4-way batch x 2-way head-group
(8 heads per core).  W_eps_attn/W_attn are column-sharded by head; y is
exchanged pair-wise per 512-token half (AllGather overlapped with the second
attention half / x_eps projection) and W_proj/W_eps_proj are used
column-sharded so each core produces a disjoint half of both outputs.

Phase order per core:  B (q/k proj + RoPE) -> C (vp proj) -> D (attention,
softmax pipelined: scores run 4 k-tiles ahead of the exp/mask chain so the
PE never waits on ScalarE) -> X/V output projections (collectives hidden
under X).  All matmuls bf16 with fp32 PSUM accumulation.

Scheduling notes baked into the structure:
 - every weight/activation tensor is stored partition-major in DRAM
   ([128, n*free]) so each stream is a handful of large 2D DMAs (8 KiB
   per-partition lines) instead of hundreds of small dispatches;
 - RoPE does 4 PSUM-reading multiplies on VectorE + 2 SBUF folds on GpSimdE
   (no PSUM->SBUF staging copies), halving DVE load vs a naive rope;
 - softmax uses a single ones-matmul for the denominator (replicated over
   partitions by the PE) and reciprocal_approx_fast (~5x cheaper than the
   exact DVE reciprocal; z is O(1)..O(1e5), far from its edge cases);
 - outputs are written bf16 (host upcasts) halving the output DMA.
"""

import sys

sys.path.insert(0, "/opt/trn_rl_repo")

import numpy as np
import ml_dtypes

import concourse.bass as bass
import concourse.mybir as mybir
import concourse.tile as tile
from concourse import bacc
from concourse.bass_utils import run_bass_kernel_spmd

F32 = mybir.dt.float32
BF16 = mybir.dt.bfloat16
BF16_NP = ml_dtypes.bfloat16

B, T, DIM, H, HD = 4, 1024, 2048, 16, 128
NCORES = 8
HL = H // 2          # heads per core (8)
TT = T // 128        # token tiles (8)
CC = DIM // 128      # contraction chunks (16)
QC = T // 512        # 512-token halves (2)
COLS = DIM // 2      # local output columns (1024)

_COMPILED = None


def _build():
    nc = bacc.Bacc(trn_type="TRN2", target_bir_lowering=False, debug=False,
                   num_devices=NCORES)

    MUL = mybir.AluOpType.mult
    SUB = mybir.AluOpType.subtract
    ADD = mybir.AluOpType.add

    # ---- per-core I/O: everything partition-major [128, n*free] ----
    x_in = nc.dram_tensor("x_in", [128, CC * T], BF16, kind="ExternalInput").ap()
    v_in = nc.dram_tensor("v_in", [128, CC * T], BF16, kind="ExternalInput").ap()
    cos_in = nc.dram_tensor("cosd", [128, T], BF16, kind="ExternalInput").ap()
    sin_in = nc.dram_tensor("sind", [128, T], BF16, kind="ExternalInput").ap()
    wqk_in = nc.dram_tensor("w_qk", [8, 128, CC * 256], BF16,
                            kind="ExternalInput").ap()
    wat_in = nc.dram_tensor("w_attn", [128, CC * COLS], BF16,
                            kind="ExternalInput").ap()
    wpr_in = nc.dram_tensor("w_proj", [128, CC * COLS], BF16,
                            kind="ExternalInput").ap()
    wep_in = nc.dram_tensor("w_eps_proj", [128, CC * COLS], BF16,
                            kind="ExternalInput").ap()
    v_out = nc.dram_tensor("v_out", [128, TT * COLS], BF16,
                           kind="ExternalOutput").ap()
    x_out = nc.dram_tensor("x_out", [128, TT * COLS], BF16,
                           kind="ExternalOutput").ap()

    # internal DRAM for the chunked pair-wise y exchange
    y_bounce = [nc.dram_tensor(f"y_bounce{qc}", [128, HL * 512], BF16)
                for qc in range(QC)]
    y_gather = [nc.dram_tensor(f"y_gather{qc}", [2, 128, HL * 512], BF16)
                for qc in range(QC)]

    with tile.TileContext(nc) as tc:
        with tc.tile_pool(name="pp", bufs=1) as pp, \
             tc.tile_pool(name="rp", bufs=2) as rp:

            xT = pp.tile([128, CC * T], BF16, tag="xT", name="xT")
            wprT = pp.tile([128, CC * COLS], BF16, tag="wpr", name="wprT")
            ones_mat = pp.tile([128, 128], BF16, tag="ones", name="ones_mat")
            nc.vector.memset(ones_mat[:], 1.0)

            # first thing on the sync HWDGE queue: the chunk stage B needs
            nc.sync.dma_start(xT[:, 0:T], x_in[:, 0:T])

            with tc.tile_pool(name="bdp", bufs=1) as bdp:
                qkT = bdp.tile([128, 2 * HL * T], BF16, tag="qkT", name="qkT")
                vp = bdp.tile([128, TT * COLS], BF16, tag="vp", name="vp")
                # causal 0/1 masks, variant m: keep iff q_rel - k_rel - 128m >= 0
                masks = []
                for m in range(4):
                    mk = bdp.tile([128, 512], BF16, tag=f"mask{m}", name=f"mask{m}")
                    nc.gpsimd.memset(mk[:], 1.0)
                    nc.gpsimd.affine_select(
                        out=mk[:], in_=mk[:], compare_op=mybir.AluOpType.is_ge,
                        fill=0.0, base=-128 * m, pattern=[[1, 512]],
                        channel_multiplier=-1)
                    masks.append(mk)

                with tc.tile_pool(name="vtp", bufs=1) as vtp:
                    vT = vtp.tile([128, CC * T], BF16, tag="vT", name="vT")

                    # ---- stage B: q/k projection + RoPE ----
                    with tc.tile_pool(name="wqkp", bufs=2) as wqkp, \
                         tc.tile_pool(name="rtp", bufs=2) as rtp, \
                         tc.tile_pool(name="cstp", bufs=1) as cstp, \
                         tc.tile_pool(name="psA", bufs=8, space="PSUM") as psA:

                        # cos/sin duplicated into both partition halves so every
                        # RoPE multiply pairs equal base partitions
                        cosD = cstp.tile([128, T], BF16, tag="cosD", name="cosD")
                        nc.gpsimd.dma_start(cosD[:], cos_in)
                        sinD = cstp.tile([128, T], BF16, tag="sinD", name="sinD")
                        nc.gpsimd.dma_start(sinD[:], sin_in)

                        for G in range(8):
                            wt = wqkp.tile([128, CC * 256], BF16, tag="wqk",
                                           name=f"wqk{G}")
                            if G == 0:
                                # split G0's load so the first matmul only
                                # waits on chunks c0/c1
                                nc.sync.dma_start(wt[:, 0:512], wqk_in[0][:, 0:512])
                                nc.sync.dma_start(wt[:, 512:], wqk_in[0][:, 512:])
                                nc.sync.dma_start(xT[:, T:4 * T], x_in[:, T:4 * T])
                                nc.sync.dma_start(xT[:, 4 * T:10 * T],
                                                  x_in[:, 4 * T:10 * T])
                                nc.sync.dma_start(xT[:, 10 * T:], x_in[:, 10 * T:])
                            else:
                                nc.sync.dma_start(wt[:], wqk_in[G])
                                if G == 1:
                                    for i in range(4):
                                        s = slice(i * 4 * T, (i + 1) * 4 * T)
                                        nc.sync.dma_start(vT[:, s], v_in[:, s])
                                    half = CC * COLS // 2
                                    nc.sync.dma_start(wprT[:, 0:half],
                                                      wpr_in[:, 0:half])
                                    nc.sync.dma_start(wprT[:, half:],
                                                      wpr_in[:, half:])

                            psg = [[psA.tile([128, 512], F32, tag="ps",
                                             name=f"psB{G}_{dl}_{q}")
                                    for q in range(QC)] for dl in range(2)]
                            for c in range(CC):
                                for dl in range(2):
                                    w_sl = wt[:, c * 256 + dl * 128:
                                              c * 256 + (dl + 1) * 128]
                                    for q in range(QC):
                                        nc.tensor.matmul(
                                            psg[dl][q][:], w_sl,
                                            xT[:, c * T + q * 512:
                                               c * T + q * 512 + 512],
                                            start=(c == 0), stop=(c == CC - 1))
                            # RoPE: psum rows 0:64 = re, 64:128 = im
                            for dl in range(2):
                                dt = 2 * G + dl
                                for q in range(QC):
                                    ps = psg[dl][q]
                                    cs = slice(q * 512, (q + 1) * 512)
                                    oc = slice(dt * T + q * 512,
                                               dt * T + q * 512 + 512)
                                    t1 = rtp.tile([64, 512], BF16, tag="t1")
                                    nc.vector.tensor_tensor(
                                        t1[:], ps[0:64, :], cosD[0:64, cs], MUL)
                                    t2 = rtp.tile([64, 512], BF16, tag="t2")
                                    nc.vector.tensor_tensor(
                                        t2[:], ps[64:128, :], sinD[64:128, cs], MUL)
                                    nc.gpsimd.tensor_tensor(
                                        qkT[0:64, oc], t1[:], t2[:], SUB)
                                    t3 = rtp.tile([64, 512], BF16, tag="t3")
                                    nc.vector.tensor_tensor(
                                        t3[:], ps[0:64, :], sinD[0:64, cs], MUL)
                                    t4 = rtp.tile([64, 512], BF16, tag="t4")
                                    nc.vector.tensor_tensor(
                                        t4[:], ps[64:128, :], cosD[64:128, cs], MUL)
                                    nc.gpsimd.tensor_tensor(
                                        qkT[64:128, oc], t3[:], t4[:], ADD)

                        # ---- stage C: vp = v @ W_attn (both token halves) ----
                        with tc.tile_pool(name="watp", bufs=2) as watp:
                            for qcv in range(QC):
                                psg = [[psA.tile([128, 512], F32, tag="ps",
                                                 name=f"psC{qcv}_{tl}_{dh}")
                                        for dh in range(2)] for tl in range(4)]
                                for cb in range(4):
                                    wtb = watp.tile([128, 4 * COLS], BF16,
                                                    tag="wat", name=f"wat{qcv}_{cb}")
                                    nc.sync.dma_start(
                                        wtb[:], wat_in[:, cb * 4 * COLS:
                                                       (cb + 1) * 4 * COLS])
                                    for ci in range(4):
                                        c = cb * 4 + ci
                                        for tl in range(4):
                                            t = qcv * 4 + tl
                                            v_sl = vT[:, c * T + t * 128:
                                                      c * T + t * 128 + 128]
                                            for dh in range(2):
                                                nc.tensor.matmul(
                                                    psg[tl][dh][:], v_sl,
                                                    wtb[:, ci * COLS + dh * 512:
                                                        ci * COLS + dh * 512 + 512],
                                                    start=(c == 0),
                                                    stop=(c == CC - 1))
                                for tl in range(4):
                                    t = qcv * 4 + tl
                                    for dh in range(2):
                                        nc.vector.tensor_copy(
                                            vp[:, t * COLS + dh * 512:
                                               t * COLS + dh * 512 + 512],
                                            psg[tl][dh][:])

                # ---- stage D: causal attention, softmax pipelined ----
                # scores run 4 k-tiles ahead; ScalarE exp is the bottleneck
                # engine, PE fills with y/z accumulation + lookahead scores.
                with tc.tile_pool(name="ytbp", bufs=2) as ytbp, \
                     tc.tile_pool(name="ptp", bufs=4) as ptp, \
                     tc.tile_pool(name="psD", bufs=4, space="PSUM") as psD:
                    ytb = [ytbp.tile([128, HL * 512], BF16, tag="ytb",
                                     name=f"ytb{qc}") for qc in range(QC)]
                    for qc in range(QC):
                        K = 4 * (qc + 1)
                        for j in range(HL):
                            kbase = (HL + j) * T
                            qsl = qkT[:, j * T + qc * 512: j * T + qc * 512 + 512]
                            scq = {}

                            def emit_sc(ki, qc=qc, j=j, kbase=kbase, qsl=qsl,
                                        scq=scq):
                                s = psD.tile([128, 512], F32, tag="pscr",
                                             name=f"sc{qc}_{j}_{ki}")
                                nc.tensor.matmul(
                                    s[:],
                                    qkT[:, kbase + ki * 128: kbase + (ki + 1) * 128],
                                    qsl, start=True, stop=True)
                                scq[ki] = s

                            for ki in range(min(4, K)):
                                emit_sc(ki)
                            py = psD.tile([128, 512], F32, tag="pypz",
                                          name=f"py{qc}_{j}")
                            pz = psD.tile([128, 512], F32, tag="pypz",
                                          name=f"pz{qc}_{j}")
                            for ki in range(K):
                                pt = ptp.tile([128, 512], BF16, tag="pt",
                                              name=f"pt{qc}_{j}_{ki}")
                                nc.scalar.activation(
                                    pt[:], scq.pop(ki)[:],
                                    mybir.ActivationFunctionType.Exp)
                                if ki >= 4 * qc:
                                    nc.vector.tensor_tensor(
                                        pt[:], pt[:], masks[ki - 4 * qc][:], MUL)
                                nc.tensor.matmul(
                                    py[:],
                                    vp[:, ki * COLS + j * 128:
                                       ki * COLS + (j + 1) * 128],
                                    pt[:], start=(ki == 0), stop=(ki == K - 1))
                                nc.tensor.matmul(
                                    pz[:], ones_mat[:], pt[:],
                                    start=(ki == 0), stop=(ki == K - 1))
                                if ki + 4 < K:
                                    emit_sc(ki + 4)
                            zr = rp.tile([128, 512], F32, tag="zr",
                                         name=f"zr{qc}_{j}")
                            nc.vector.reciprocal_approx_fast(zr[:], pz[:])
                            nc.vector.tensor_tensor(
                                ytb[qc][:, j * 512:(j + 1) * 512],
                                py[:], zr[:], MUL)
                        # ship this half: own 8 heads -> DRAM -> pair AllGather
                        nc.gpsimd.dma_start(y_bounce[qc].ap(), ytb[qc][:])
                        nc.gpsimd.collective_compute(
                            "AllGather", mybir.AluOpType.bypass,
                            replica_groups=[[0, 1], [2, 3], [4, 5], [6, 7]],
                            ins=[y_bounce[qc].ap()], outs=[y_gather[qc].ap()])

            # ---- stages X/V: output projections (exchange hidden under X) ----
            with tc.tile_pool(name="xvp", bufs=2) as xvp, \
                 tc.tile_pool(name="wepp", bufs=2) as wepp, \
                 tc.tile_pool(name="ocp", bufs=2) as ocp, \
                 tc.tile_pool(name="psXV", bufs=8, space="PSUM") as psXV:

                yG2 = [xvp.tile([128, 2 * HL * 512], BF16, tag="yg",
                                name=f"yg{qc}") for qc in range(QC)]
                for qc in range(QC):
                    for r in range(2):
                        nc.gpsimd.dma_start(
                            yG2[qc][:, r * HL * 512:(r + 1) * HL * 512],
                            y_gather[qc].ap()[r])

                for tg in range(QC):
                    # x_eps_out rows [tg*512, ..+512): no exchange dependency
                    psg = [[psXV.tile([128, 512], F32, tag="ps",
                                      name=f"psX{tg}_{tl}_{dh}")
                            for dh in range(2)] for tl in range(4)]
                    for cb in range(4):
                        wtb = wepp.tile([128, 4 * COLS], BF16, tag="wep",
                                        name=f"wep{tg}_{cb}")
                        nc.sync.dma_start(
                            wtb[:], wep_in[:, cb * 4 * COLS:(cb + 1) * 4 * COLS])
                        for ci in range(4):
                            c = cb * 4 + ci
                            for tl in range(4):
                                t = tg * 4 + tl
                                x_sl = xT[:, c * T + t * 128: c * T + t * 128 + 128]
                                for dh in range(2):
                                    nc.tensor.matmul(
                                        psg[tl][dh][:], x_sl,
                                        wtb[:, ci * COLS + dh * 512:
                                            ci * COLS + dh * 512 + 512],
                                        start=(c == 0), stop=(c == CC - 1))
                    og = ocp.tile([128, 4 * COLS], BF16, tag="og", name=f"ox{tg}")
                    for tl in range(4):
                        for dh in range(2):
                            nc.vector.tensor_copy(
                                og[:, tl * COLS + dh * 512:
                                   tl * COLS + dh * 512 + 512],
                                psg[tl][dh][:])
                    nc.sync.dma_start(
                        x_out[:, tg * 4 * COLS:(tg + 1) * 4 * COLS], og[:])

                    # v_out rows for the same token half (needs exchange tg)
                    psg = [[psXV.tile([128, 512], F32, tag="ps",
                                      name=f"psV{tg}_{tl}_{dh}")
                            for dh in range(2)] for tl in range(4)]
                    for c in range(CC):
                        for tl in range(4):
                            y_sl = yG2[tg][:, c * 512 + tl * 128:
                                           c * 512 + tl * 128 + 128]
                            for dh in range(2):
                                nc.tensor.matmul(
                                    psg[tl][dh][:], y_sl,
                                    wprT[:, c * COLS + dh * 512:
                                         c * COLS + dh * 512 + 512],
                                    start=(c == 0), stop=(c == CC - 1))
                    og = ocp.tile([128, 4 * COLS], BF16, tag="og", name=f"ov{tg}")
                    for tl in range(4):
                        for dh in range(2):
                            nc.vector.tensor_copy(
                                og[:, tl * COLS + dh * 512:
                                   tl * COLS + dh * 512 + 512],
                                psg[tl][dh][:])
                    nc.sync.dma_start(
                        v_out[:, tg * 4 * COLS:(tg + 1) * 4 * COLS], og[:])

    nc.compile()
    return nc


def _prep_core_inputs(inputs):
    """Host-side shard prep: slicing, bf16 cast, partition-major packing."""
    x_eps = np.asarray(inputs["x_eps"], np.float32)
    v = np.asarray(inputs["v"], np.float32)
    cos = np.asarray(inputs["freqs_cos"], np.float32)
    sin = np.asarray(inputs["freqs_sin"], np.float32)
    Wqk = np.asarray(inputs["W_eps_attn"], np.float32)
    Wat = np.asarray(inputs["W_attn"], np.float32)
    Wpr = np.asarray(inputs["W_proj"], np.float32)
    Wep = np.asarray(inputs["W_eps_proj"], np.float32)

    cosD = np.ascontiguousarray(
        np.concatenate([cos.T, cos.T], axis=0)).astype(BF16_NP)   # [128, T]
    sinD = np.ascontiguousarray(
        np.concatenate([sin.T, sin.T], axis=0)).astype(BF16_NP)
    perm = np.concatenate([np.arange(0, HD, 2), np.arange(1, HD, 2)])  # re|im
    scale = np.float32(1.0 / np.sqrt(HD))

    def pm_act(a):  # [T, DIM] fp32 -> [128, CC*T] bf16, col = c*T + t
        return np.ascontiguousarray(
            a.astype(BF16_NP).T.reshape(CC, 128, T)
            .transpose(1, 0, 2).reshape(128, CC * T))

    xT_bf = [pm_act(x_eps[b]) for b in range(B)]
    vT_bf = [pm_act(v[b]) for b in range(B)]

    def pm_w(Wc):  # [DIM, COLS] fp32 -> [128, CC*COLS] bf16, col = c*COLS + f
        return np.ascontiguousarray(
            Wc.reshape(CC, 128, COLS).transpose(1, 0, 2)
            .reshape(128, CC * COLS).astype(BF16_NP))

    per_g = []
    for g in range(2):
        heads = range(g * HL, (g + 1) * HL)
        wq = np.concatenate(
            [Wqk[:, h * HD:(h + 1) * HD][:, perm] * scale for h in heads], axis=1)
        wk = np.concatenate(
            [Wqk[:, DIM + h * HD:DIM + (h + 1) * HD][:, perm] for h in heads],
            axis=1)
        wqk_cols = np.concatenate([wq, wk], axis=1)          # [2048, 2048]
        # [G, p, c*256 + dl*128 + col]
        wqk_p = np.ascontiguousarray(
            wqk_cols.reshape(CC, 128, 8, 2, 128)
            .transpose(2, 1, 0, 3, 4).reshape(8, 128, CC * 256)).astype(BF16_NP)
        cols = slice(g * COLS, (g + 1) * COLS)
        per_g.append({
            "w_qk": wqk_p,
            "w_attn": pm_w(Wat[:, cols]),
            "w_proj": pm_w(Wpr[:, cols]),
            "w_eps_proj": pm_w(Wep[:, cols]),
        })

    in_maps = []
    for core in range(NCORES):
        b, g = divmod(core, 2)
        in_maps.append({
            "x_in": xT_bf[b],
            "v_in": vT_bf[b],
            "cosd": cosD,
            "sind": sinD,
            **per_g[g],
        })
    return in_maps


def _get_compiled():
    global _COMPILED
    if _COMPILED is None:
        _COMPILED = _build()
    return _COMPILED


def kernel(**inputs):
    nc = _get_compiled()
    in_maps = _prep_core_inputs(inputs)
    res = run_bass_kernel_spmd(nc, in_maps, list(range(NCORES)))
    v_full = np.empty((B, T, DIM), np.float32)
    x_full = np.empty((B, T, DIM), np.float32)
    for core in range(NCORES):
        b, g = divmod(core, 2)
        r = res.results[core]
        cols = slice(g * COLS, (g + 1) * COLS)
        vo = np.asarray(r["v_out"]).reshape(128, TT, COLS).transpose(1, 0, 2)
        xo = np.asarray(r["x_out"]).reshape(128, TT, COLS).transpose(1, 0, 2)
        v_full[b][:, cols] = vo.reshape(T, COLS).astype(np.float32)
        x_full[b][:, cols] = xo.reshape(T, COLS).astype(np.float32)
    return (v_full, x_full)


# revision 8
# speedup vs baseline: 1.2649x; 1.0771x over previous
"""Distributed Trainium2 (Bass/Tile) kernel for a causal self-attention block.

Reference computation (per batch b):
    qk = x_eps @ W_eps_attn ; q,k = split(qk) ; vp = v @ W_attn
    q,k = rope(q), rope(k)   (llama-style, 16 heads x 128 dims)
    y   = causal_softmax(q k^T / sqrt(128)) @ vp   (per head)
    v_out     = y @ W_proj
    x_eps_out = x_eps @ W_eps_proj

Sharding over 8 NeuronCores: core = (b, g) = 4-way batch x 2-way head-group
(8 heads per core).  W_eps_attn/W_attn are column-sharded by head; y is
exchanged pair-wise per 512-token half (AllGather overlapped with the second
attention half / x_eps projection) and W_proj/W_eps_proj are used
column-sharded so each core produces a disjoint half of both outputs.

Phase order per core:  B (q/k proj + RoPE) -> C (vp proj) -> D (attention,
softmax pipelined: scores run 4 k-tiles ahead of the exp/mask chain so the
PE never waits on ScalarE) -> X/V output projections (collectives hidden
under X).  All matmuls bf16 with fp32 PSUM accumulation.

Scheduling notes baked into the structure:
 - every weight/activation tensor is stored partition-major in DRAM
   ([128, n*free]) so each stream is a handful of large 2D DMAs (8 KiB
   per-partition lines) instead of hundreds of small dispatches;
 - RoPE does 4 PSUM-reading multiplies on VectorE + 2 SBUF folds on GpSimdE
   (no PSUM->SBUF staging copies), halving DVE load vs a naive rope;
 - softmax uses a single ones-matmul for the denominator (replicated over
   partitions by the PE) and reciprocal_approx_fast (~5x cheaper than the
   exact DVE reciprocal; z is O(1)..O(1e5), far from its edge cases);
 - outputs are written bf16 (host upcasts) halving the output DMA.
"""

import sys

sys.path.insert(0, "/opt/trn_rl_repo")

import numpy as np
import ml_dtypes

import concourse.bass as bass
import concourse.mybir as mybir
import concourse.tile as tile
from concourse import bacc
from concourse.bass_utils import run_bass_kernel_spmd

F32 = mybir.dt.float32
BF16 = mybir.dt.bfloat16
BF16_NP = ml_dtypes.bfloat16

B, T, DIM, H, HD = 4, 1024, 2048, 16, 128
NCORES = 8
HL = H // 2          # heads per core (8)
TT = T // 128        # token tiles (8)
CC = DIM // 128      # contraction chunks (16)
QC = T // 512        # 512-token halves (2)
COLS = DIM // 2      # local output columns (1024)

_COMPILED = None


def _build():
    nc = bacc.Bacc(trn_type="TRN2", target_bir_lowering=False, debug=False,
                   num_devices=NCORES)

    MUL = mybir.AluOpType.mult
    SUB = mybir.AluOpType.subtract
    ADD = mybir.AluOpType.add

    # ---- per-core I/O: everything partition-major [128, n*free] ----
    x_in = nc.dram_tensor("x_in", [128, CC * T], BF16, kind="ExternalInput").ap()
    v_in = nc.dram_tensor("v_in", [128, CC * T], BF16, kind="ExternalInput").ap()
    cos_in = nc.dram_tensor("cosd", [128, T], BF16, kind="ExternalInput").ap()
    sin_in = nc.dram_tensor("sind", [128, T], BF16, kind="ExternalInput").ap()
    wqk_in = nc.dram_tensor("w_qk", [8, 128, CC * 256], BF16,
                            kind="ExternalInput").ap()
    wat_in = nc.dram_tensor("w_attn", [128, CC * COLS], BF16,
                            kind="ExternalInput").ap()
    wpr_in = nc.dram_tensor("w_proj", [128, CC * COLS], BF16,
                            kind="ExternalInput").ap()
    wep_in = nc.dram_tensor("w_eps_proj", [128, CC * COLS], BF16,
                            kind="ExternalInput").ap()
    v_out = nc.dram_tensor("v_out", [128, TT * COLS], BF16,
                           kind="ExternalOutput").ap()
    x_out = nc.dram_tensor("x_out", [128, TT * COLS], BF16,
                           kind="ExternalOutput").ap()

    # internal DRAM for the chunked pair-wise y exchange
    y_bounce = [nc.dram_tensor(f"y_bounce{qc}", [128, HL * 512], BF16)
                for qc in range(QC)]
    y_gather = [nc.dram_tensor(f"y_gather{qc}", [2, 128, HL * 512], BF16)
                for qc in range(QC)]

    with tile.TileContext(nc) as tc:
        with tc.tile_pool(name="pp", bufs=1) as pp, \
             tc.tile_pool(name="rp", bufs=2) as rp:

            xT = pp.tile([128, CC * T], BF16, tag="xT", name="xT")
            wprT = pp.tile([128, CC * COLS], BF16, tag="wpr", name="wprT")
            # vT doubles as the W_eps_proj buffer once stage C has drained it
            vT = pp.tile([128, CC * T], BF16, tag="vT", name="vT")
            ones_mat = pp.tile([128, 128], BF16, tag="ones", name="ones_mat")
            nc.vector.memset(ones_mat[:], 1.0)

            # first thing on the sync HWDGE queue: the chunk stage B needs
            nc.sync.dma_start(xT[:, 0:T], x_in[:, 0:T])

            with tc.tile_pool(name="bdp", bufs=1) as bdp:
                qkT = bdp.tile([128, 2 * HL * T], BF16, tag="qkT", name="qkT")
                vp = bdp.tile([128, TT * COLS], BF16, tag="vp", name="vp")
                # causal 0/1 masks, variant m: keep iff q_rel - k_rel - 128m >= 0
                masks = []
                for m in range(4):
                    mk = bdp.tile([128, 512], BF16, tag=f"mask{m}", name=f"mask{m}")
                    nc.gpsimd.memset(mk[:], 1.0)
                    nc.gpsimd.affine_select(
                        out=mk[:], in_=mk[:], compare_op=mybir.AluOpType.is_ge,
                        fill=0.0, base=-128 * m, pattern=[[1, 512]],
                        channel_multiplier=-1)
                    masks.append(mk)

                if True:
                    # ---- stage B: q/k projection + RoPE ----
                    with tc.tile_pool(name="wqkp", bufs=2) as wqkp, \
                         tc.tile_pool(name="rtp", bufs=2) as rtp, \
                         tc.tile_pool(name="cstp", bufs=1) as cstp, \
                         tc.tile_pool(name="psA", bufs=8, space="PSUM") as psA:

                        # cos/sin duplicated into both partition halves so every
                        # RoPE multiply pairs equal base partitions
                        cosD = cstp.tile([128, T], BF16, tag="cosD", name="cosD")
                        nc.gpsimd.dma_start(cosD[:], cos_in)
                        sinD = cstp.tile([128, T], BF16, tag="sinD", name="sinD")
                        nc.gpsimd.dma_start(sinD[:], sin_in)

                        for G in range(8):
                            wt = wqkp.tile([128, CC * 256], BF16, tag="wqk",
                                           name=f"wqk{G}")
                            if G == 0:
                                # split G0's load so the first matmuls only
                                # wait on the chunks they consume
                                nc.sync.dma_start(wt[:, 0:512], wqk_in[0][:, 0:512])
                                nc.sync.dma_start(xT[:, T:4 * T], x_in[:, T:4 * T])
                                nc.sync.dma_start(wt[:, 512:1536],
                                                  wqk_in[0][:, 512:1536])
                                nc.sync.dma_start(wt[:, 1536:],
                                                  wqk_in[0][:, 1536:])
                                nc.sync.dma_start(xT[:, 4 * T:10 * T],
                                                  x_in[:, 4 * T:10 * T])
                                nc.sync.dma_start(xT[:, 10 * T:], x_in[:, 10 * T:])
                            else:
                                nc.sync.dma_start(wt[:], wqk_in[G])
                                # vT arrives late in B, just ahead of stage C
                                if G in (5, 6):
                                    for i in range(2):
                                        ib = (2 * (G - 5) + i) * 4 * T
                                        s = slice(ib, ib + 4 * T)
                                        nc.sync.dma_start(vT[:, s], v_in[:, s])

                            psg = [[psA.tile([128, 512], F32, tag="ps",
                                             name=f"psB{G}_{dl}_{q}")
                                    for q in range(QC)] for dl in range(2)]
                            for c in range(CC):
                                for dl in range(2):
                                    w_sl = wt[:, c * 256 + dl * 128:
                                              c * 256 + (dl + 1) * 128]
                                    for q in range(QC):
                                        nc.tensor.matmul(
                                            psg[dl][q][:], w_sl,
                                            xT[:, c * T + q * 512:
                                               c * T + q * 512 + 512],
                                            start=(c == 0), stop=(c == CC - 1))
                            # RoPE: psum rows 0:64 = re, 64:128 = im
                            for dl in range(2):
                                dt = 2 * G + dl
                                for q in range(QC):
                                    ps = psg[dl][q]
                                    cs = slice(q * 512, (q + 1) * 512)
                                    oc = slice(dt * T + q * 512,
                                               dt * T + q * 512 + 512)
                                    t1 = rtp.tile([64, 512], BF16, tag="t1")
                                    nc.vector.tensor_tensor(
                                        t1[:], ps[0:64, :], cosD[0:64, cs], MUL)
                                    t2 = rtp.tile([64, 512], BF16, tag="t2")
                                    nc.vector.tensor_tensor(
                                        t2[:], ps[64:128, :], sinD[64:128, cs], MUL)
                                    nc.gpsimd.tensor_tensor(
                                        qkT[0:64, oc], t1[:], t2[:], SUB)
                                    t3 = rtp.tile([64, 512], BF16, tag="t3")
                                    nc.vector.tensor_tensor(
                                        t3[:], ps[0:64, :], sinD[0:64, cs], MUL)
                                    t4 = rtp.tile([64, 512], BF16, tag="t4")
                                    nc.vector.tensor_tensor(
                                        t4[:], ps[64:128, :], cosD[64:128, cs], MUL)
                                    nc.gpsimd.tensor_tensor(
                                        qkT[64:128, oc], t3[:], t4[:], ADD)

                        # ---- stage C: vp = v @ W_attn (both token halves) ----
                        with tc.tile_pool(name="watp", bufs=2) as watp:
                            for qcv in range(QC):
                                psg = [[psA.tile([128, 512], F32, tag="ps",
                                                 name=f"psC{qcv}_{tl}_{dh}")
                                        for dh in range(2)] for tl in range(4)]
                                for cb in range(4):
                                    wtb = watp.tile([128, 4 * COLS], BF16,
                                                    tag="wat", name=f"wat{qcv}_{cb}")
                                    nc.sync.dma_start(
                                        wtb[:], wat_in[:, cb * 4 * COLS:
                                                       (cb + 1) * 4 * COLS])
                                    for ci in range(4):
                                        c = cb * 4 + ci
                                        for tl in range(4):
                                            t = qcv * 4 + tl
                                            v_sl = vT[:, c * T + t * 128:
                                                      c * T + t * 128 + 128]
                                            for dh in range(2):
                                                nc.tensor.matmul(
                                                    psg[tl][dh][:], v_sl,
                                                    wtb[:, ci * COLS + dh * 512:
                                                        ci * COLS + dh * 512 + 512],
                                                    start=(c == 0),
                                                    stop=(c == CC - 1))
                                for tl in range(4):
                                    t = qcv * 4 + tl
                                    for dh in range(2):
                                        nc.vector.tensor_copy(
                                            vp[:, t * COLS + dh * 512:
                                               t * COLS + dh * 512 + 512],
                                            psg[tl][dh][:])

                # queue W_proj and W_eps_proj behind the attention weights:
                # both transfer during stage D, before the collectives start
                # competing for DMA bandwidth.  W_eps_proj reuses vT's space
                # (stage C has fully consumed v by now).
                half = CC * COLS // 2
                nc.sync.dma_start(wprT[:, 0:half], wpr_in[:, 0:half])
                nc.sync.dma_start(wprT[:, half:], wpr_in[:, half:])
                nc.sync.dma_start(vT[:, 0:half], wep_in[:, 0:half])
                nc.sync.dma_start(vT[:, half:], wep_in[:, half:])

                # ---- stage D: causal attention, softmax pipelined ----
                # scores run 4 k-tiles ahead; ScalarE exp is the bottleneck
                # engine, PE fills with y/z accumulation + lookahead scores.
                with tc.tile_pool(name="ytbp", bufs=2) as ytbp, \
                     tc.tile_pool(name="ptp", bufs=4) as ptp, \
                     tc.tile_pool(name="psD", bufs=4, space="PSUM") as psD:
                    ytb = [ytbp.tile([128, HL * 512], BF16, tag="ytb",
                                     name=f"ytb{qc}") for qc in range(QC)]
                    for qc in range(QC):
                        K = 4 * (qc + 1)
                        for j in range(HL):
                            kbase = (HL + j) * T
                            qsl = qkT[:, j * T + qc * 512: j * T + qc * 512 + 512]
                            scq = {}

                            def emit_sc(ki, qc=qc, j=j, kbase=kbase, qsl=qsl,
                                        scq=scq):
                                s = psD.tile([128, 512], F32, tag="pscr",
                                             name=f"sc{qc}_{j}_{ki}")
                                nc.tensor.matmul(
                                    s[:],
                                    qkT[:, kbase + ki * 128: kbase + (ki + 1) * 128],
                                    qsl, start=True, stop=True)
                                scq[ki] = s

                            for ki in range(min(4, K)):
                                emit_sc(ki)
                            py = psD.tile([128, 512], F32, tag="pypz",
                                          name=f"py{qc}_{j}")
                            pz = psD.tile([128, 512], F32, tag="pypz",
                                          name=f"pz{qc}_{j}")
                            for ki in range(K):
                                pt = ptp.tile([128, 512], BF16, tag="pt",
                                              name=f"pt{qc}_{j}_{ki}")
                                nc.scalar.activation(
                                    pt[:], scq.pop(ki)[:],
                                    mybir.ActivationFunctionType.Exp)
                                if ki >= 4 * qc:
                                    nc.vector.tensor_tensor(
                                        pt[:], pt[:], masks[ki - 4 * qc][:], MUL)
                                nc.tensor.matmul(
                                    py[:],
                                    vp[:, ki * COLS + j * 128:
                                       ki * COLS + (j + 1) * 128],
                                    pt[:], start=(ki == 0), stop=(ki == K - 1))
                                nc.tensor.matmul(
                                    pz[:], ones_mat[:], pt[:],
                                    start=(ki == 0), stop=(ki == K - 1))
                                if ki + 4 < K:
                                    emit_sc(ki + 4)
                            zr = rp.tile([128, 512], F32, tag="zr",
                                         name=f"zr{qc}_{j}")
                            nc.vector.reciprocal_approx_fast(zr[:], pz[:])
                            nc.vector.tensor_tensor(
                                ytb[qc][:, j * 512:(j + 1) * 512],
                                py[:], zr[:], MUL)
                        # ship this half: own 8 heads -> DRAM -> pair AllGather
                        nc.gpsimd.dma_start(y_bounce[qc].ap(), ytb[qc][:])
                        nc.gpsimd.collective_compute(
                            "AllGather", mybir.AluOpType.bypass,
                            replica_groups=[[0, 1], [2, 3], [4, 5], [6, 7]],
                            ins=[y_bounce[qc].ap()], outs=[y_gather[qc].ap()])

            # ---- stages X/V: output projections (exchange hidden under X) ----
            with tc.tile_pool(name="xvp", bufs=2) as xvp, \
                 tc.tile_pool(name="ocp", bufs=2) as ocp, \
                 tc.tile_pool(name="psXV", bufs=8, space="PSUM") as psXV:

                yG2 = [xvp.tile([128, 2 * HL * 512], BF16, tag="yg",
                                name=f"yg{qc}") for qc in range(QC)]
                for qc in range(QC):
                    for r in range(2):
                        nc.gpsimd.dma_start(
                            yG2[qc][:, r * HL * 512:(r + 1) * HL * 512],
                            y_gather[qc].ap()[r])

                def drain(psg, og):
                    # split the PSUM->SBUF casts across DVE and ACT
                    for tl in range(4):
                        for dh in range(2):
                            dst = og[:, tl * COLS + dh * 512:
                                     tl * COLS + dh * 512 + 512]
                            if dh == 0:
                                nc.vector.tensor_copy(dst, psg[tl][dh][:])
                            else:
                                nc.scalar.copy(dst, psg[tl][dh][:])

                for tg in range(QC):
                    # x_eps_out rows [tg*512, ..+512): no exchange dependency
                    psg = [[psXV.tile([128, 512], F32, tag="ps",
                                      name=f"psX{tg}_{tl}_{dh}")
                            for dh in range(2)] for tl in range(4)]
                    for c in range(CC):
                        for tl in range(4):
                            t = tg * 4 + tl
                            x_sl = xT[:, c * T + t * 128: c * T + t * 128 + 128]
                            for dh in range(2):
                                nc.tensor.matmul(
                                    psg[tl][dh][:], x_sl,
                                    vT[:, c * COLS + dh * 512:
                                       c * COLS + dh * 512 + 512],
                                    start=(c == 0), stop=(c == CC - 1))
                    og = ocp.tile([128, 4 * COLS], BF16, tag="og", name=f"ox{tg}")
                    drain(psg, og)
                    nc.sync.dma_start(
                        x_out[:, tg * 4 * COLS:(tg + 1) * 4 * COLS], og[:])

                    # v_out rows for the same token half (needs exchange tg)
                    psg = [[psXV.tile([128, 512], F32, tag="ps",
                                      name=f"psV{tg}_{tl}_{dh}")
                            for dh in range(2)] for tl in range(4)]
                    for c in range(CC):
                        for tl in range(4):
                            y_sl = yG2[tg][:, c * 512 + tl * 128:
                                           c * 512 + tl * 128 + 128]
                            for dh in range(2):
                                nc.tensor.matmul(
                                    psg[tl][dh][:], y_sl,
                                    wprT[:, c * COLS + dh * 512:
                                         c * COLS + dh * 512 + 512],
                                    start=(c == 0), stop=(c == CC - 1))
                    og = ocp.tile([128, 4 * COLS], BF16, tag="og", name=f"ov{tg}")
                    drain(psg, og)
                    nc.sync.dma_start(
                        v_out[:, tg * 4 * COLS:(tg + 1) * 4 * COLS], og[:])

    nc.compile()
    return nc


def _prep_core_inputs(inputs):
    """Host-side shard prep: slicing, bf16 cast, partition-major packing."""
    x_eps = np.asarray(inputs["x_eps"], np.float32)
    v = np.asarray(inputs["v"], np.float32)
    cos = np.asarray(inputs["freqs_cos"], np.float32)
    sin = np.asarray(inputs["freqs_sin"], np.float32)
    Wqk = np.asarray(inputs["W_eps_attn"], np.float32)
    Wat = np.asarray(inputs["W_attn"], np.float32)
    Wpr = np.asarray(inputs["W_proj"], np.float32)
    Wep = np.asarray(inputs["W_eps_proj"], np.float32)

    cosD = np.ascontiguousarray(
        np.concatenate([cos.T, cos.T], axis=0)).astype(BF16_NP)   # [128, T]
    sinD = np.ascontiguousarray(
        np.concatenate([sin.T, sin.T], axis=0)).astype(BF16_NP)
    perm = np.concatenate([np.arange(0, HD, 2), np.arange(1, HD, 2)])  # re|im
    scale = np.float32(1.0 / np.sqrt(HD))

    def pm_act(a):  # [T, DIM] fp32 -> [128, CC*T] bf16, col = c*T + t
        return np.ascontiguousarray(
            a.astype(BF16_NP).T.reshape(CC, 128, T)
            .transpose(1, 0, 2).reshape(128, CC * T))

    xT_bf = [pm_act(x_eps[b]) for b in range(B)]
    vT_bf = [pm_act(v[b]) for b in range(B)]

    def pm_w(Wc):  # [DIM, COLS] fp32 -> [128, CC*COLS] bf16, col = c*COLS + f
        return np.ascontiguousarray(
            Wc.reshape(CC, 128, COLS).transpose(1, 0, 2)
            .reshape(128, CC * COLS).astype(BF16_NP))

    per_g = []
    for g in range(2):
        heads = range(g * HL, (g + 1) * HL)
        wq = np.concatenate(
            [Wqk[:, h * HD:(h + 1) * HD][:, perm] * scale for h in heads], axis=1)
        wk = np.concatenate(
            [Wqk[:, DIM + h * HD:DIM + (h + 1) * HD][:, perm] for h in heads],
            axis=1)
        wqk_cols = np.concatenate([wq, wk], axis=1)          # [2048, 2048]
        # [G, p, c*256 + dl*128 + col]
        wqk_p = np.ascontiguousarray(
            wqk_cols.reshape(CC, 128, 8, 2, 128)
            .transpose(2, 1, 0, 3, 4).reshape(8, 128, CC * 256)).astype(BF16_NP)
        cols = slice(g * COLS, (g + 1) * COLS)
        per_g.append({
            "w_qk": wqk_p,
            "w_attn": pm_w(Wat[:, cols]),
            "w_proj": pm_w(Wpr[:, cols]),
            "w_eps_proj": pm_w(Wep[:, cols]),
        })

    in_maps = []
    for core in range(NCORES):
        b, g = divmod(core, 2)
        in_maps.append({
            "x_in": xT_bf[b],
            "v_in": vT_bf[b],
            "cosd": cosD,
            "sind": sinD,
            **per_g[g],
        })
    return in_maps


def _get_compiled():
    global _COMPILED
    if _COMPILED is None:
        _COMPILED = _build()
    return _COMPILED


def kernel(**inputs):
    nc = _get_compiled()
    in_maps = _prep_core_inputs(inputs)
    res = run_bass_kernel_spmd(nc, in_maps, list(range(NCORES)))
    v_full = np.empty((B, T, DIM), np.float32)
    x_full = np.empty((B, T, DIM), np.float32)
    for core in range(NCORES):
        b, g = divmod(core, 2)
        r = res.results[core]
        cols = slice(g * COLS, (g + 1) * COLS)
        vo = np.asarray(r["v_out"]).reshape(128, TT, COLS).transpose(1, 0, 2)
        xo = np.asarray(r["x_out"]).reshape(128, TT, COLS).transpose(1, 0, 2)
        v_full[b][:, cols] = vo.reshape(T, COLS).astype(np.float32)
        x_full[b][:, cols] = xo.reshape(T, COLS).astype(np.float32)
    return (v_full, x_full)


# revision 13
# speedup vs baseline: 1.2880x; 1.0183x over previous
"""Distributed Trainium2 (Bass/Tile) kernel for a causal self-attention block.

Reference computation (per batch b):
    qk = x_eps @ W_eps_attn ; q,k = split(qk) ; vp = v @ W_attn
    q,k = rope(q), rope(k)   (llama-style, 16 heads x 128 dims)
    y   = causal_softmax(q k^T / sqrt(128)) @ vp   (per head)
    v_out     = y @ W_proj
    x_eps_out = x_eps @ W_eps_proj

Sharding over 8 NeuronCores: core = (b, g) = 4-way batch x 2-way head-group
(8 heads per core).  W_eps_attn/W_attn are column-sharded by head; y is
exchanged pair-wise per 512-token half (AllGather overlapped with the second
attention half / x_eps projection) and W_proj/W_eps_proj are used
column-sharded so each core produces a disjoint half of both outputs.

Phase order per core:  B (q/k proj + RoPE) -> C (vp proj) -> D (attention,
softmax pipelined: scores run 4 k-tiles ahead of the exp/mask chain so the
PE never waits on ScalarE) -> X/V output projections (collectives hidden
under X).  All matmuls bf16 with fp32 PSUM accumulation.

Scheduling notes baked into the structure:
 - every weight/activation tensor is stored partition-major in DRAM
   ([128, n*free]) so each stream is a handful of large 2D DMAs (8 KiB
   per-partition lines) instead of hundreds of small dispatches;
 - RoPE does 4 PSUM-reading multiplies on VectorE + 2 SBUF folds on GpSimdE
   (no PSUM->SBUF staging copies), halving DVE load vs a naive rope;
 - softmax uses a single ones-matmul for the denominator (replicated over
   partitions by the PE) and reciprocal_approx_fast (~5x cheaper than the
   exact DVE reciprocal; z is O(1)..O(1e5), far from its edge cases);
 - outputs are written bf16 (host upcasts) halving the output DMA.
"""

import sys

sys.path.insert(0, "/opt/trn_rl_repo")

import numpy as np
import ml_dtypes

import concourse.bass as bass
import concourse.mybir as mybir
import concourse.tile as tile
from concourse import bacc
from concourse.bass_utils import run_bass_kernel_spmd

F32 = mybir.dt.float32
BF16 = mybir.dt.bfloat16
BF16_NP = ml_dtypes.bfloat16

B, T, DIM, H, HD = 4, 1024, 2048, 16, 128
NCORES = 8
HL = H // 2          # heads per core (8)
TT = T // 128        # token tiles (8)
CC = DIM // 128      # contraction chunks (16)
QC = T // 512        # 512-token halves (2)
COLS = DIM // 2      # local output columns (1024)

_COMPILED = None


def _build():
    nc = bacc.Bacc(trn_type="TRN2", target_bir_lowering=False, debug=False,
                   num_devices=NCORES)

    MUL = mybir.AluOpType.mult
    SUB = mybir.AluOpType.subtract
    ADD = mybir.AluOpType.add

    # ---- per-core I/O: everything partition-major [128, n*free] ----
    x_in = nc.dram_tensor("x_in", [128, CC * T], BF16, kind="ExternalInput").ap()
    v_in = nc.dram_tensor("v_in", [128, CC * T], BF16, kind="ExternalInput").ap()
    cos_in = nc.dram_tensor("cosd", [128, T], BF16, kind="ExternalInput").ap()
    sin_in = nc.dram_tensor("sind", [128, T], BF16, kind="ExternalInput").ap()
    wqk_in = nc.dram_tensor("w_qk", [8, 128, CC * 256], BF16,
                            kind="ExternalInput").ap()
    wat_in = nc.dram_tensor("w_attn", [128, CC * COLS], BF16,
                            kind="ExternalInput").ap()
    wpr_in = nc.dram_tensor("w_proj", [128, CC * COLS], BF16,
                            kind="ExternalInput").ap()
    wep_in = nc.dram_tensor("w_eps_proj", [128, CC * COLS], BF16,
                            kind="ExternalInput").ap()
    v_out = nc.dram_tensor("v_out", [128, TT * COLS], BF16,
                           kind="ExternalOutput").ap()
    x_out = nc.dram_tensor("x_out", [128, TT * COLS], BF16,
                           kind="ExternalOutput").ap()

    # internal DRAM for the chunked pair-wise y exchange
    y_bounce = [nc.dram_tensor(f"y_bounce{qc}", [128, HL * 512], BF16)
                for qc in range(QC)]
    y_gather = [nc.dram_tensor(f"y_gather{qc}", [2, 128, HL * 512], BF16)
                for qc in range(QC)]

    with tile.TileContext(nc) as tc:
        with tc.tile_pool(name="pp", bufs=1) as pp, \
             tc.tile_pool(name="rp", bufs=2) as rp:

            xT = pp.tile([128, CC * T], BF16, tag="xT", name="xT")
            wprT = pp.tile([128, CC * COLS], BF16, tag="wpr", name="wprT")
            # vT doubles as the W_eps_proj buffer once stage C has drained it
            vT = pp.tile([128, CC * T], BF16, tag="vT", name="vT")
            ones_mat = pp.tile([128, 128], BF16, tag="ones", name="ones_mat")
            nc.vector.memset(ones_mat[:], 1.0)

            # first thing on the sync HWDGE queue: the chunk stage B needs
            nc.sync.dma_start(xT[:, 0:T], x_in[:, 0:T])

            # PE warm-up: ~3.5us of matmul activity while the first input
            # DMAs are in flight trips the HAM clock gate to 8/8 before the
            # real work starts (the DMA to scratch keeps it from being DCEd)
            wu_scratch = nc.dram_tensor("wu_scratch", [128, 512], F32)
            wu_in = pp.tile([128, 512], BF16, tag="wu_in", name="wu_in")
            nc.vector.memset(wu_in[:], 0.0)

            with tc.tile_pool(name="bdp", bufs=1) as bdp:
                qkT = bdp.tile([128, 2 * HL * T], BF16, tag="qkT", name="qkT")
                vp = bdp.tile([128, TT * COLS], BF16, tag="vp", name="vp")
                # causal 0/1 masks, variant m: keep iff q_rel - k_rel - 128m >= 0
                masks = []
                for m in range(4):
                    mk = bdp.tile([128, 512], BF16, tag=f"mask{m}", name=f"mask{m}")
                    nc.gpsimd.memset(mk[:], 1.0)
                    nc.gpsimd.affine_select(
                        out=mk[:], in_=mk[:], compare_op=mybir.AluOpType.is_ge,
                        fill=0.0, base=-128 * m, pattern=[[1, 512]],
                        channel_multiplier=-1)
                    masks.append(mk)

                if True:
                    # ---- stage B: q/k projection + RoPE ----
                    with tc.tile_pool(name="wqkp", bufs=2) as wqkp, \
                         tc.tile_pool(name="rtp", bufs=2) as rtp, \
                         tc.tile_pool(name="cstp", bufs=1) as cstp, \
                         tc.tile_pool(name="psA", bufs=8, space="PSUM") as psA:

                        # cos/sin duplicated into both partition halves so every
                        # RoPE multiply pairs equal base partitions
                        cosD = cstp.tile([128, T], BF16, tag="cosD", name="cosD")
                        nc.gpsimd.dma_start(cosD[:], cos_in)
                        sinD = cstp.tile([128, T], BF16, tag="sinD", name="sinD")
                        nc.gpsimd.dma_start(sinD[:], sin_in)

                        pwu = psA.tile([128, 512], F32, tag="ps", name="pwu")
                        for i in range(10):
                            nc.tensor.matmul(pwu[:], ones_mat[:], wu_in[:],
                                             start=(i == 0), stop=(i == 9))
                        wu_out = rtp.tile([128, 512], F32, tag="wu_out")
                        nc.vector.tensor_copy(wu_out[:], pwu[:])
                        nc.sync.dma_start(wu_scratch.ap(), wu_out[:])

                        for G in range(8):
                            wt = wqkp.tile([128, CC * 256], BF16, tag="wqk",
                                           name=f"wqk{G}")
                            if G == 0:
                                # interleave G0's weight/activation pieces so
                                # the bandwidth-limited ramp stalls stay small
                                nc.sync.dma_start(wt[:, 0:512], wqk_in[0][:, 0:512])
                                nc.sync.dma_start(xT[:, T:3 * T], x_in[:, T:3 * T])
                                nc.sync.dma_start(wt[:, 512:1024],
                                                  wqk_in[0][:, 512:1024])
                                nc.sync.dma_start(xT[:, 3 * T:6 * T],
                                                  x_in[:, 3 * T:6 * T])
                                nc.sync.dma_start(wt[:, 1024:2048],
                                                  wqk_in[0][:, 1024:2048])
                                nc.sync.dma_start(xT[:, 6 * T:10 * T],
                                                  x_in[:, 6 * T:10 * T])
                                nc.sync.dma_start(wt[:, 2048:3072],
                                                  wqk_in[0][:, 2048:3072])
                                nc.sync.dma_start(xT[:, 10 * T:13 * T],
                                                  x_in[:, 10 * T:13 * T])
                                nc.sync.dma_start(wt[:, 3072:],
                                                  wqk_in[0][:, 3072:])
                                nc.sync.dma_start(xT[:, 13 * T:], x_in[:, 13 * T:])
                            else:
                                nc.sync.dma_start(wt[:], wqk_in[G])
                                # vT arrives late in B, just ahead of stage C
                                if G in (5, 6):
                                    for i in range(2):
                                        ib = (2 * (G - 5) + i) * 4 * T
                                        s = slice(ib, ib + 4 * T)
                                        nc.sync.dma_start(vT[:, s], v_in[:, s])

                            psg = [[psA.tile([128, 512], F32, tag="ps",
                                             name=f"psB{G}_{dl}_{q}")
                                    for q in range(QC)] for dl in range(2)]
                            for c in range(CC):
                                for dl in range(2):
                                    w_sl = wt[:, c * 256 + dl * 128:
                                              c * 256 + (dl + 1) * 128]
                                    for q in range(QC):
                                        nc.tensor.matmul(
                                            psg[dl][q][:], w_sl,
                                            xT[:, c * T + q * 512:
                                               c * T + q * 512 + 512],
                                            start=(c == 0), stop=(c == CC - 1))
                            # RoPE: psum rows 0:64 = re, 64:128 = im
                            for dl in range(2):
                                dt = 2 * G + dl
                                for q in range(QC):
                                    ps = psg[dl][q]
                                    cs = slice(q * 512, (q + 1) * 512)
                                    oc = slice(dt * T + q * 512,
                                               dt * T + q * 512 + 512)
                                    t1 = rtp.tile([64, 512], BF16, tag="t1")
                                    nc.vector.tensor_tensor(
                                        t1[:], ps[0:64, :], cosD[0:64, cs], MUL)
                                    t2 = rtp.tile([64, 512], BF16, tag="t2")
                                    nc.vector.tensor_tensor(
                                        t2[:], ps[64:128, :], sinD[64:128, cs], MUL)
                                    nc.gpsimd.tensor_tensor(
                                        qkT[0:64, oc], t1[:], t2[:], SUB)
                                    t3 = rtp.tile([64, 512], BF16, tag="t3")
                                    nc.vector.tensor_tensor(
                                        t3[:], ps[0:64, :], sinD[0:64, cs], MUL)
                                    t4 = rtp.tile([64, 512], BF16, tag="t4")
                                    nc.vector.tensor_tensor(
                                        t4[:], ps[64:128, :], cosD[64:128, cs], MUL)
                                    nc.gpsimd.tensor_tensor(
                                        qkT[64:128, oc], t3[:], t4[:], ADD)

                        # ---- stage C: vp = v @ W_attn (both token halves) ----
                        with tc.tile_pool(name="watp", bufs=2) as watp:
                            for qcv in range(QC):
                                psg = [[psA.tile([128, 512], F32, tag="ps",
                                                 name=f"psC{qcv}_{tl}_{dh}")
                                        for dh in range(2)] for tl in range(4)]
                                for cb in range(4):
                                    wtb = watp.tile([128, 4 * COLS], BF16,
                                                    tag="wat", name=f"wat{qcv}_{cb}")
                                    nc.sync.dma_start(
                                        wtb[:], wat_in[:, cb * 4 * COLS:
                                                       (cb + 1) * 4 * COLS])
                                    for ci in range(4):
                                        c = cb * 4 + ci
                                        for tl in range(4):
                                            t = qcv * 4 + tl
                                            v_sl = vT[:, c * T + t * 128:
                                                      c * T + t * 128 + 128]
                                            for dh in range(2):
                                                nc.tensor.matmul(
                                                    psg[tl][dh][:], v_sl,
                                                    wtb[:, ci * COLS + dh * 512:
                                                        ci * COLS + dh * 512 + 512],
                                                    start=(c == 0),
                                                    stop=(c == CC - 1))
                                # drain split across DVE/ACT so the PSUM banks
                                # hand off to the next phase twice as fast
                                for tl in range(4):
                                    t = qcv * 4 + tl
                                    for dh in range(2):
                                        dst = vp[:, t * COLS + dh * 512:
                                                 t * COLS + dh * 512 + 512]
                                        if dh == 0:
                                            nc.vector.tensor_copy(
                                                dst, psg[tl][dh][:])
                                        else:
                                            nc.scalar.copy(dst, psg[tl][dh][:])

                # queue W_proj and W_eps_proj behind the attention weights:
                # both transfer during stage D, before the collectives start
                # competing for DMA bandwidth.  W_eps_proj reuses vT's space
                # (stage C has fully consumed v by now).
                half = CC * COLS // 2
                nc.sync.dma_start(wprT[:, 0:half], wpr_in[:, 0:half])
                nc.sync.dma_start(wprT[:, half:], wpr_in[:, half:])
                nc.sync.dma_start(vT[:, 0:half], wep_in[:, 0:half])
                nc.sync.dma_start(vT[:, half:], wep_in[:, half:])

                # ---- stage D: causal attention, softmax pipelined ----
                # scores run 4 k-tiles ahead; ScalarE exp is the bottleneck
                # engine, PE fills with y/z accumulation + lookahead scores.
                with tc.tile_pool(name="ytbp", bufs=2) as ytbp, \
                     tc.tile_pool(name="ptp", bufs=4) as ptp, \
                     tc.tile_pool(name="psD", bufs=4, space="PSUM") as psD:
                    ytb = [ytbp.tile([128, HL * 512], BF16, tag="ytb",
                                     name=f"ytb{qc}") for qc in range(QC)]
                    for qc in range(QC):
                        K = 4 * (qc + 1)
                        for j in range(HL):
                            kbase = (HL + j) * T
                            qsl = qkT[:, j * T + qc * 512: j * T + qc * 512 + 512]
                            scq = {}

                            def emit_sc(ki, qc=qc, j=j, kbase=kbase, qsl=qsl,
                                        scq=scq):
                                s = psD.tile([128, 512], F32, tag="pscr",
                                             name=f"sc{qc}_{j}_{ki}")
                                nc.tensor.matmul(
                                    s[:],
                                    qkT[:, kbase + ki * 128: kbase + (ki + 1) * 128],
                                    qsl, start=True, stop=True)
                                scq[ki] = s

                            for ki in range(min(4, K)):
                                emit_sc(ki)
                            py = psD.tile([128, 512], F32, tag="pypz",
                                          name=f"py{qc}_{j}")
                            pz = psD.tile([128, 512], F32, tag="pypz",
                                          name=f"pz{qc}_{j}")
                            for ki in range(K):
                                pt = ptp.tile([128, 512], BF16, tag="pt",
                                              name=f"pt{qc}_{j}_{ki}")
                                nc.scalar.activation(
                                    pt[:], scq.pop(ki)[:],
                                    mybir.ActivationFunctionType.Exp)
                                if ki >= 4 * qc:
                                    nc.vector.tensor_tensor(
                                        pt[:], pt[:], masks[ki - 4 * qc][:], MUL)
                                nc.tensor.matmul(
                                    py[:],
                                    vp[:, ki * COLS + j * 128:
                                       ki * COLS + (j + 1) * 128],
                                    pt[:], start=(ki == 0), stop=(ki == K - 1))
                                nc.tensor.matmul(
                                    pz[:], ones_mat[:], pt[:],
                                    start=(ki == 0), stop=(ki == K - 1))
                                if ki + 4 < K:
                                    emit_sc(ki + 4)
                            zr = rp.tile([128, 512], F32, tag="zr",
                                         name=f"zr{qc}_{j}")
                            nc.vector.reciprocal_approx_fast(zr[:], pz[:])
                            nc.vector.tensor_tensor(
                                ytb[qc][:, j * 512:(j + 1) * 512],
                                py[:], zr[:], MUL)
                        # ship this half: own 8 heads -> DRAM -> pair AllGather
                        nc.gpsimd.dma_start(y_bounce[qc].ap(), ytb[qc][:])
                        nc.gpsimd.collective_compute(
                            "AllGather", mybir.AluOpType.bypass,
                            replica_groups=[[0, 1], [2, 3], [4, 5], [6, 7]],
                            ins=[y_bounce[qc].ap()], outs=[y_gather[qc].ap()])

            # ---- stages X/V: output projections (exchange hidden under X) ----
            with tc.tile_pool(name="xvp", bufs=2) as xvp, \
                 tc.tile_pool(name="ocp", bufs=2) as ocp, \
                 tc.tile_pool(name="psXV", bufs=8, space="PSUM") as psXV:

                yG2 = [xvp.tile([128, 2 * HL * 512], BF16, tag="yg",
                                name=f"yg{qc}") for qc in range(QC)]
                for qc in range(QC):
                    for r in range(2):
                        nc.gpsimd.dma_start(
                            yG2[qc][:, r * HL * 512:(r + 1) * HL * 512],
                            y_gather[qc].ap()[r])

                def drain(psg, og):
                    # split the PSUM->SBUF casts across DVE and ACT
                    for tl in range(4):
                        for dh in range(2):
                            dst = og[:, tl * COLS + dh * 512:
                                     tl * COLS + dh * 512 + 512]
                            if dh == 0:
                                nc.vector.tensor_copy(dst, psg[tl][dh][:])
                            else:
                                nc.scalar.copy(dst, psg[tl][dh][:])

                for tg in range(QC):
                    # x_eps_out rows [tg*512, ..+512): no exchange dependency
                    psg = [[psXV.tile([128, 512], F32, tag="ps",
                                      name=f"psX{tg}_{tl}_{dh}")
                            for dh in range(2)] for tl in range(4)]
                    for c in range(CC):
                        for tl in range(4):
                            t = tg * 4 + tl
                            x_sl = xT[:, c * T + t * 128: c * T + t * 128 + 128]
                            for dh in range(2):
                                nc.tensor.matmul(
                                    psg[tl][dh][:], x_sl,
                                    vT[:, c * COLS + dh * 512:
                                       c * COLS + dh * 512 + 512],
                                    start=(c == 0), stop=(c == CC - 1))
                    og = ocp.tile([128, 4 * COLS], BF16, tag="og", name=f"ox{tg}")
                    drain(psg, og)
                    nc.sync.dma_start(
                        x_out[:, tg * 4 * COLS:(tg + 1) * 4 * COLS], og[:])

                    # v_out rows for the same token half (needs exchange tg)
                    psg = [[psXV.tile([128, 512], F32, tag="ps",
                                      name=f"psV{tg}_{tl}_{dh}")
                            for dh in range(2)] for tl in range(4)]
                    for c in range(CC):
                        for tl in range(4):
                            y_sl = yG2[tg][:, c * 512 + tl * 128:
                                           c * 512 + tl * 128 + 128]
                            for dh in range(2):
                                nc.tensor.matmul(
                                    psg[tl][dh][:], y_sl,
                                    wprT[:, c * COLS + dh * 512:
                                         c * COLS + dh * 512 + 512],
                                    start=(c == 0), stop=(c == CC - 1))
                    og = ocp.tile([128, 4 * COLS], BF16, tag="og", name=f"ov{tg}")
                    drain(psg, og)
                    nc.sync.dma_start(
                        v_out[:, tg * 4 * COLS:(tg + 1) * 4 * COLS], og[:])

    nc.compile()
    return nc


def _prep_core_inputs(inputs):
    """Host-side shard prep: slicing, bf16 cast, partition-major packing."""
    x_eps = np.asarray(inputs["x_eps"], np.float32)
    v = np.asarray(inputs["v"], np.float32)
    cos = np.asarray(inputs["freqs_cos"], np.float32)
    sin = np.asarray(inputs["freqs_sin"], np.float32)
    Wqk = np.asarray(inputs["W_eps_attn"], np.float32)
    Wat = np.asarray(inputs["W_attn"], np.float32)
    Wpr = np.asarray(inputs["W_proj"], np.float32)
    Wep = np.asarray(inputs["W_eps_proj"], np.float32)

    cosD = np.ascontiguousarray(
        np.concatenate([cos.T, cos.T], axis=0)).astype(BF16_NP)   # [128, T]
    sinD = np.ascontiguousarray(
        np.concatenate([sin.T, sin.T], axis=0)).astype(BF16_NP)
    perm = np.concatenate([np.arange(0, HD, 2), np.arange(1, HD, 2)])  # re|im
    scale = np.float32(1.0 / np.sqrt(HD))

    def pm_act(a):  # [T, DIM] fp32 -> [128, CC*T] bf16, col = c*T + t
        return np.ascontiguousarray(
            a.astype(BF16_NP).T.reshape(CC, 128, T)
            .transpose(1, 0, 2).reshape(128, CC * T))

    xT_bf = [pm_act(x_eps[b]) for b in range(B)]
    vT_bf = [pm_act(v[b]) for b in range(B)]

    def pm_w(Wc):  # [DIM, COLS] fp32 -> [128, CC*COLS] bf16, col = c*COLS + f
        return np.ascontiguousarray(
            Wc.reshape(CC, 128, COLS).transpose(1, 0, 2)
            .reshape(128, CC * COLS).astype(BF16_NP))

    per_g = []
    for g in range(2):
        heads = range(g * HL, (g + 1) * HL)
        wq = np.concatenate(
            [Wqk[:, h * HD:(h + 1) * HD][:, perm] * scale for h in heads], axis=1)
        wk = np.concatenate(
            [Wqk[:, DIM + h * HD:DIM + (h + 1) * HD][:, perm] for h in heads],
            axis=1)
        wqk_cols = np.concatenate([wq, wk], axis=1)          # [2048, 2048]
        # [G, p, c*256 + dl*128 + col]
        wqk_p = np.ascontiguousarray(
            wqk_cols.reshape(CC, 128, 8, 2, 128)
            .transpose(2, 1, 0, 3, 4).reshape(8, 128, CC * 256)).astype(BF16_NP)
        cols = slice(g * COLS, (g + 1) * COLS)
        per_g.append({
            "w_qk": wqk_p,
            "w_attn": pm_w(Wat[:, cols]),
            "w_proj": pm_w(Wpr[:, cols]),
            "w_eps_proj": pm_w(Wep[:, cols]),
        })

    in_maps = []
    for core in range(NCORES):
        b, g = divmod(core, 2)
        in_maps.append({
            "x_in": xT_bf[b],
            "v_in": vT_bf[b],
            "cosd": cosD,
            "sind": sinD,
            **per_g[g],
        })
    return in_maps


def _get_compiled():
    global _COMPILED
    if _COMPILED is None:
        _COMPILED = _build()
    return _COMPILED


def kernel(**inputs):
    nc = _get_compiled()
    in_maps = _prep_core_inputs(inputs)
    res = run_bass_kernel_spmd(nc, in_maps, list(range(NCORES)))
    v_full = np.empty((B, T, DIM), np.float32)
    x_full = np.empty((B, T, DIM), np.float32)
    for core in range(NCORES):
        b, g = divmod(core, 2)
        r = res.results[core]
        cols = slice(g * COLS, (g + 1) * COLS)
        vo = np.asarray(r["v_out"]).reshape(128, TT, COLS).transpose(1, 0, 2)
        xo = np.asarray(r["x_out"]).reshape(128, TT, COLS).transpose(1, 0, 2)
        v_full[b][:, cols] = vo.reshape(T, COLS).astype(np.float32)
        x_full[b][:, cols] = xo.reshape(T, COLS).astype(np.float32)
    return (v_full, x_full)
